# revision 29
# baseline (speedup 1.0000x reference)
"""Boundary BCE loss kernel for Trainium2 (8 NeuronCores, data-parallel).

Computes mean(BCEWithLogits(pred, boundary(gt_mask))) where
boundary(m) = 1 iff the 3x3 neighborhood of a pixel (SAME window, valid
elements only) contains both a 0 and a 1.

Key identities / layout tricks:
  - With *replicate* padding the value-set of a 3x3 window equals the set of
    valid in-bounds values, so boundary <=> 0 < s < 9, where s = replicate-pad
    3x3 weighted count of ones (weights sum to 9; s is an exact small integer).
  - The whole 3x3 conv runs on the tensor engine: a banded [K,M] bf16
    stationary matrix A does the vertical taps, and the three horizontal taps
    come from *column-shifted* copies of the same moving operand accumulated
    in PSUM: s[:, c] = sum_d A^T @ mf[:, c+d], d in {-1,0,1}, with the two
    replicate edge columns supplied by tiny N=1 matmuls from a 2-column tile
    g (cast straight from gt on GpSimd). Every tile has exactly ONE writer --
    a second writer would serialize the conv chain across engines.
  - Input row-blocks of 128 overlap by 2 rows so no halo fixups exist; the
    top/bottom replicate rows are folded into A. The 8 images' ragged bottom
    strips (16 rows) are *stacked* into one full [128, W] block via a 3D DMA
    access pattern and a block-diagonal A -- every block is full-height.
  - elementwise loss = softplus(x) - x*z  (z = boundary in {0,1})
    sum(loss) = sum(ln(1+exp(x))) - [sum(x*(s>=0.5)) - sum(x*(s>=8.5))]
    (x ~ N(0,1) so exp(x) cannot overflow). Exp/Ln share one ACT table set
    (natural_log_exp_and_others -- see _patch_act_tables); Ln's free affine
    adds the +1; all three sums ride fused accum_out ports (ACT, DVE stt).
  - All loads are exactly 128-partition DMAs on the SP HWDGE ring -- only
    128-row transfers split evenly across the 16 SDMA engines (shorter ones
    overload engine 0 ~2x). pred loads are padded to 128 rows for this.
  - Emission is software-pipelined two blocks ahead (conv-front ops enter
    each engine's in-order queue before older blocks' reduction ops) to
    avoid head-of-line blocking.

Each core reduces its 8-image shard to 3 partial vectors [128, n_blocks];
the host sums those in float64 and divides by N.
"""

import os
import sys
from collections import deque
from contextlib import ExitStack

import numpy as np

if "/opt/trn_rl_repo" not in sys.path and os.path.isdir("/opt/trn_rl_repo"):
    sys.path.append("/opt/trn_rl_repo")

N_CORES = 8
B, C, H, W = 64, 1, 1024, 1024
IMGS_PER_CORE = B // N_CORES  # 8
P = 128


def img_blocks(h):
    """Per-image row tiling: (in_r0, in_rows, out_r0, out_rows, kind)."""
    blocks = [(0, 128, 0, 127, "top")]
    out0 = 127
    while h - out0 > 126:
        blocks.append((out0 - 1, 128, out0, 126, "int"))
        out0 += 126
    m = h - out0
    blocks.append((out0 - 1, m + 1, out0, m, "bot"))
    return blocks


def make_consts(h=H, n_imgs=IMGS_PER_CORE):
    """Banded vertical-conv matrices A[k, m] = weight of input row k in out m."""
    import ml_dtypes

    bf16 = ml_dtypes.bfloat16

    atop = np.zeros((128, 127), np.float32)
    for m in range(127):
        for k in (m - 1, m, m + 1):
            if 0 <= k < 128:
                atop[k, m] += 1.0
    atop[0, 0] += 1.0  # replicate row -1 -> row 0

    aint = np.zeros((128, 126), np.float32)
    for m in range(126):
        for k in (m, m + 1, m + 2):
            aint[k, m] += 1.0

    mb = img_blocks(h)[-1][3]
    abot = np.zeros((mb + 1, mb), np.float32)
    for m in range(mb):
        for k in (m, m + 1, m + 2):
            if k <= mb:
                abot[k, m] += 1.0
    abot[mb, mb - 1] += 1.0  # replicate row h -> row h-1

    # block-diagonal stack of the per-image bottom strips
    abst = np.zeros((n_imgs * (mb + 1), n_imgs * mb), np.float32)
    for j in range(n_imgs):
        abst[j * (mb + 1) : (j + 1) * (mb + 1), j * mb : (j + 1) * mb] = abot

    return {
        "conv_atop": atop.astype(bf16),
        "conv_aint": aint.astype(bf16),
        "conv_abst": abst.astype(bf16),
    }


def build_program(nc, n_imgs=IMGS_PER_CORE, h=H, w=W):
    """Emit the per-core Tile program onto `nc` (a Bacc)."""
    import concourse.tile as tile
    from concourse import mybir

    blocks = img_blocks(h)
    full_blocks = blocks[:-1]  # per-image; bottom strips are stacked
    bot = blocks[-1]
    mb = bot[3]
    kbs = n_imgs * (mb + 1)
    mbs = n_imgs * mb
    assert kbs <= 128, (n_imgs, mb)
    n_blk = n_imgs * len(full_blocks) + 1
    rows = n_imgs * h

    f32 = mybir.dt.float32
    i32 = mybir.dt.int32
    bf16 = mybir.dt.bfloat16

    pred_d = nc.dram_tensor("pred", [rows, w], f32, kind="ExternalInput")
    gt_d = nc.dram_tensor("gt", [rows, w], i32, kind="ExternalInput")
    atop_d = nc.dram_tensor("conv_atop", [128, 127], bf16, kind="ExternalInput")
    aint_d = nc.dram_tensor("conv_aint", [128, 126], bf16, kind="ExternalInput")
    abst_d = nc.dram_tensor("conv_abst", [kbs, mbs], bf16, kind="ExternalInput")
    # partials: cols [0,n) = softplus sums, [n,2n) = x*(s>=.5), [2n,3n) = x*(s>=8.5)
    out_d = nc.dram_tensor("partials", [P, 3 * n_blk], f32, kind="ExternalOutput")

    pred = pred_d.ap()
    gt = gt_d.ap()
    pred3 = pred.rearrange("(j r) c -> j r c", j=n_imgs)
    gt3 = gt.rearrange("(j r) c -> j r c", j=n_imgs)
    out = out_d.ap()

    with tile.TileContext(nc) as tc, ExitStack() as ctx:
        consts = ctx.enter_context(tc.tile_pool(name="consts", bufs=1))
        xs = ctx.enter_context(tc.tile_pool(name="xs", bufs=10))
        gts = ctx.enter_context(tc.tile_pool(name="gts", bufs=10))
        mfs = ctx.enter_context(tc.tile_pool(name="mfs", bufs=8))
        gs = ctx.enter_context(tc.tile_pool(name="gs", bufs=8))
        scratch = ctx.enter_context(tc.tile_pool(name="scratch", bufs=4))
        accp = ctx.enter_context(tc.tile_pool(name="accs", bufs=1))
        psum = ctx.enter_context(tc.tile_pool(name="psum", bufs=4, space="PSUM"))

        atop = consts.tile([128, 127], bf16, tag="atop")
        aint = consts.tile([128, 126], bf16, tag="aint")
        abst = consts.tile([kbs, mbs], bf16, tag="abst")
        nc.sync.dma_start(atop[:], atop_d.ap()[:])
        nc.sync.dma_start(aint[:], aint_d.ap()[:])
        nc.sync.dma_start(abst[:], abst_d.ap()[:])
        a_mats = {"top": atop, "int": aint, "bst": abst}

        # one accumulator per producing engine stream so cross-engine
        # accum_out writes never alias one tile
        acc_sp = accp.tile([P, n_blk], f32, tag="acc_sp")
        acc_u = accp.tile([P, n_blk], f32, tag="acc_u")
        acc_v = accp.tile([P, n_blk], f32, tag="acc_v")
        # short blocks leave partitions >= their out_rows untouched
        nc.vector.memset(acc_sp[:], 0.0)
        nc.vector.memset(acc_u[:], 0.0)
        nc.vector.memset(acc_v[:], 0.0)

        def emit_front(gi, kind, K, M, gt_src, x_src, x_rows):
            """Conv-path ops for one block: loads, cast, edge tile, matmuls."""
            gt_t = gts.tile([K, w], i32, tag="gt")
            nc.sync.dma_start(gt_t[:], gt_src)
            x_t = xs.tile([x_rows, w], f32, tag="x")
            nc.sync.dma_start(x_t[:], x_src)

            # mf = bf16(gt), single writer (casts alternate DVE/ACT for
            # engine balance); replicate edge columns live in their own tiny
            # tile g so no tile has two writers
            mf = mfs.tile([K, w], bf16, tag="mf")
            r = gi % 5
            if r < 2:
                nc.vector.tensor_copy(mf[:], gt_t[:])
            elif r == 2:
                nc.scalar.copy(mf[:], gt_t[:])
            else:
                nc.gpsimd.tensor_copy(mf[:], gt_t[:])
            g = gs.tile([K, 2], bf16, tag="g")
            nc.gpsimd.tensor_copy(g[:], gt_t[:, 0 : w : w - 1])

            # s[:, c] = sum_k A[k,m]*(gt[c-1]+gt[c]+gt[c+1]) entirely on PE:
            # three column-shifted matmuls accumulate in PSUM; the replicate
            # taps at c=0 / c=w-1 are tiny N=1 matmuls from g
            s_ps = psum.tile([M, w], f32, tag="s")
            a = a_mats[kind]
            nc.tensor.matmul(s_ps[:, 0:512], a[:], mf[:, 0:512],
                             start=True, stop=False)
            nc.tensor.matmul(s_ps[:, 0:512], a[:], mf[:, 1:513],
                             start=False, stop=False)
            nc.tensor.matmul(s_ps[:, 1:512], a[:], mf[:, 0:511],
                             start=False, stop=False)
            nc.tensor.matmul(s_ps[:, 0:1], a[:], g[:, 0:1],
                             start=False, stop=True)
            nc.tensor.matmul(s_ps[:, 512:1024], a[:], mf[:, 512:1024],
                             start=True, stop=False)
            nc.tensor.matmul(s_ps[:, 512:1024], a[:], mf[:, 511:1023],
                             start=False, stop=False)
            nc.tensor.matmul(s_ps[:, 512:1023], a[:], mf[:, 513:1024],
                             start=False, stop=False)
            nc.tensor.matmul(s_ps[:, 1023:1024], a[:], g[:, 1:2],
                             start=False, stop=True)
            return s_ps, x_t

        def emit_back(gi, M, s_ps, x_t):
            """Reduction ops for one block: softplus sum and the x*z sums."""
            # softplus(x) = ln(1 + exp(x)); Ln's free affine adds the +1
            ex = scratch.tile([M, w], f32, tag="ex")
            nc.scalar.activation(ex[:], x_t[0:M, :], mybir.ActivationFunctionType.Exp)
            sp = scratch.tile([M, w], bf16, tag="sp")
            nc.scalar.activation(
                sp[:], ex[:], mybir.ActivationFunctionType.Ln,
                bias=1.0,
                accum_out=acc_sp[0:M, gi : gi + 1],
            )
            # sum(x * (s >= 0.5)) and sum(x * (s >= 8.5)) on DVE
            w1 = scratch.tile([M, w], bf16, tag="w1")
            nc.vector.scalar_tensor_tensor(
                w1[:], s_ps[:], 0.5, x_t[0:M, :],
                mybir.AluOpType.is_ge, mybir.AluOpType.mult,
                accum_out=acc_u[0:M, gi : gi + 1],
            )
            w2 = scratch.tile([M, w], bf16, tag="w2")
            nc.vector.scalar_tensor_tensor(
                w2[:], s_ps[:], 8.5, x_t[0:M, :],
                mybir.AluOpType.is_ge, mybir.AluOpType.mult,
                accum_out=acc_v[0:M, gi : gi + 1],
            )

        specs = []
        for img in range(n_imgs):
            for in_r0, in_rows, out_r0, out_rows, kind in full_blocks:
                ir0 = img * h + in_r0
                or0 = img * h + out_r0
                xr = min(128, rows - or0)
                specs.append((kind, in_rows, out_rows,
                              gt[ir0 : ir0 + in_rows, :],
                              pred[or0 : or0 + xr, :], xr))
        specs.append(("bst", kbs, mbs,
                      gt3[:, bot[0] : bot[0] + bot[1], :],
                      pred3[:, bot[2] : bot[2] + bot[3], :], mbs))

        pending = deque()
        for gi, (kind, K, M, gt_src, x_src, xr) in enumerate(specs):
            front = emit_front(gi, kind, K, M, gt_src, x_src, xr)
            pending.append((gi, M, front))
            if len(pending) > 2:
                pgi, pm, pf = pending.popleft()
                emit_back(pgi, pm, *pf)
        while pending:
            pgi, pm, pf = pending.popleft()
            emit_back(pgi, pm, *pf)

        nc.sync.dma_start(out[:, 0:n_blk], acc_sp[:])
        nc.sync.dma_start(out[:, n_blk : 2 * n_blk], acc_u[:])
        nc.sync.dma_start(out[:, 2 * n_blk : 3 * n_blk], acc_v[:])

    return n_blk


def _patch_act_tables():
    """Make Exp and Ln resolve to the one table set containing both
    (natural_log_exp_and_others); otherwise the table-load pass alternates
    between exp_and_others and natural_log, reloading ~1.3us per activation.
    Set indices (= positions in act_info.json's act_func_sets) are preserved;
    only the membership used for set *selection* is filtered."""
    import concourse.bacc as bacc_mod
    from concourse import mybir

    if getattr(bacc_mod, "_act_tables_patched", False):
        return
    orig = bacc_mod.get_activation_tables
    exp_ln = {mybir.ActivationFunctionType.Exp, mybir.ActivationFunctionType.Ln}

    def patched(arch):
        out = {}
        for name, fns in orig(arch).items():
            out[name] = set(fns) if name == "natural_log_exp_and_others" else (
                set(fns) - exp_ln
            )
        return out

    bacc_mod.get_activation_tables = patched
    bacc_mod._act_tables_patched = True


def _ensure_ntff_hook():
    """Best-effort: make run_bass_kernel_spmd(trace=True) usable. The agent
    container ships no antenv.axon_hooks module, so a BASS_TRACE=1 run would
    otherwise die on the import inside bass_utils. Harmless if unused."""
    try:
        import types

        import antenv

        if "antenv.axon_hooks" in sys.modules:
            return
        m = types.ModuleType("antenv.axon_hooks")
        _h = {}
        m.set_axon_ntff_profile_hook = lambda h: _h.__setitem__("h", h)
        m.get_axon_ntff_profile_hook = lambda: _h.get("h")
        sys.modules["antenv.axon_hooks"] = m
        antenv.axon_hooks = m
        try:
            from trn_agent_boot.trn_boot import _ntff_profile_via_ctypes

            so = "/opt/axon/libaxon_pjrt.so"
            if os.path.exists(so):
                m.set_axon_ntff_profile_hook(_ntff_profile_via_ctypes(so))
        except Exception:
            pass
        try:
            import concourse.bass_utils as bu

            bu.upload_artifacts = lambda tmpdir: tmpdir
        except Exception:
            pass
    except Exception:
        pass


_CACHE = {}


def _get_nc():
    if "nc" not in _CACHE:
        import concourse.bacc as bacc

        _ensure_ntff_hook()
        _patch_act_tables()
        nc = bacc.Bacc("TRN2", target_bir_lowering=False, debug=False,
                       num_devices=N_CORES)
        n_blk = build_program(nc)
        nc.compile()
        _CACHE["nc"] = nc
        _CACHE["n_blk"] = n_blk
    return _CACHE["nc"], _CACHE["n_blk"]


def kernel(pred_boundary: np.ndarray, gt_mask: np.ndarray) -> np.ndarray:
    from concourse.bass_utils import run_bass_kernel_spmd

    nc, n_blk = _get_nc()
    consts = make_consts()

    pred = np.ascontiguousarray(pred_boundary, dtype=np.float32).reshape(B * H, W)
    gt = np.ascontiguousarray(gt_mask, dtype=np.int32).reshape(B * H, W)

    rows_per_core = IMGS_PER_CORE * H
    in_maps = []
    for c in range(N_CORES):
        r0 = c * rows_per_core
        in_maps.append(
            {
                "pred": pred[r0 : r0 + rows_per_core],
                "gt": gt[r0 : r0 + rows_per_core],
                **consts,
            }
        )

    res = run_bass_kernel_spmd(nc, in_maps, list(range(N_CORES)))
    _CACHE["last_results"] = res

    total = np.float64(0.0)
    for c in range(N_CORES):
        p = res.results[c]["partials"].astype(np.float64)
        sp = p[:, 0:n_blk].sum()
        xu = p[:, n_blk : 2 * n_blk].sum()
        xv = p[:, 2 * n_blk : 3 * n_blk].sum()
        total += sp - (xu - xv)

    mean = total / float(B * C * H * W)
    return np.float32(mean)
